# revision 37
# baseline (speedup 1.0000x reference)
"""Trainium2 Bass kernel for the GNN message-passing Convolution problem.

Strategy (8 NeuronCores, SPMD):
  - Host: sort edges by destination node; shard destination nodes 8 ways
    (6250/core); within a core, group edges into bins of 128 consecutive
    dst nodes, padded to a uniform number of 128-edge tiles per bin so the
    single SPMD program works for every core.  Inputs are packed into just
    three device tensors to minimize dispatch/staging overhead:
      bigT  bf16 [32, npad + npad/2 + s_total]: node features (transposed),
            node attrs (split into two 16-row halves), slotted edge features
      epack i32 [128, ntile]: per-edge (dst_offset << 20) | src_row
      wpack f32 [128, 656]: all small weight/constant matrices
  - Device, per core:
      Phase N: node linears x = in*attr*W_lin1, s = in*attr*W_sc in
        transposed land (features on partitions) with PE matmuls (bf16);
        x rows transposed back and written to DRAM as bf16.
      AllGather x shards -> full bf16 x table.
      Phase E: per bin: one batched indirect-DMA gather of all the bin's
        x[src] rows; per 128-edge tile: radial MLP on PE, bilinear message
        as ONE broadcast tensor_tensor product [128, 8i x 256] + one
        pair-sum, then 4 accumulating one-hot scatter matmuls into PSUM.
      Phase F: per bin: lin2 via one broadcast product + reduce over o,
        add self-connection, DMA out (bf16).
  - Host: concatenate the 8 node shards, convert to f32.
"""

import math
import sys

import numpy as np

if "/opt/trn_rl_repo" not in sys.path:
    sys.path.insert(0, "/opt/trn_rl_repo")

import concourse.bacc as bacc
import concourse.mybir as mybir
from concourse.bass import IndirectOffsetOnAxis
from concourse.bass_utils import run_bass_kernel_spmd
from concourse.masks import make_identity
from concourse.tile import TileContext

F32 = mybir.dt.float32
BF16 = mybir.dt.bfloat16
I32 = mybir.dt.int32

NCORES = 8
C_S = math.sin(math.pi / 8.0)
C_X = math.cos(math.pi / 8.0)
INV_SQRT_NEI = 1.0 / math.sqrt(8.0)

# wpack layout: (row0, col0) destinations for each constant
_WPACK_COLS = 656


def _to_bf16(x):
    import ml_dtypes

    return np.asarray(x).astype(ml_dtypes.bfloat16)


# ---------------------------------------------------------------- host prep
def _host_prep(inputs, ns, nbin):
    """Build per-core input maps. ns = dst nodes per core, nbin = node bin size."""
    node_input = np.ascontiguousarray(inputs["node_input"], np.float32)  # [N,4,8]
    node_attr = np.ascontiguousarray(inputs["node_attr"], np.float32)  # [N,16]
    edge_feat = np.ascontiguousarray(inputs["edge_features"], np.float32)  # [E,32]
    W_sc = np.asarray(inputs["W_sc"], np.float32)  # [8,16,8]
    W_lin1 = np.asarray(inputs["W_lin1"], np.float32)  # [8,16,8]
    W_lin2 = np.asarray(inputs["W_lin2"], np.float32)  # [8,16,8]
    fc_w1 = np.asarray(inputs["fc_w1"], np.float32)  # [32,64]
    fc_b1 = np.asarray(inputs["fc_b1"], np.float32)  # [64]
    fc_w2 = np.asarray(inputs["fc_w2"], np.float32)  # [64,512]
    fc_b2 = np.asarray(inputs["fc_b2"], np.float32)  # [512]
    src = np.asarray(inputs["edge_src"], np.int32)
    dst = np.asarray(inputs["edge_dst"], np.int32)

    nb = (ns + nbin - 1) // nbin  # bins per core
    npad = nb * nbin  # padded nodes per core
    nh = npad // 2

    # --- per-core edge binning (uniform tiles/bin across all cores) ---
    core_of = dst // ns
    local_dst = dst - core_of * ns
    bin_of = local_dst // nbin
    counts = np.zeros((NCORES, nb), np.int64)
    np.add.at(counts, (core_of, bin_of), 1)
    tiles_per_bin = int(-(-counts.max() // 128))
    slots_per_bin = tiles_per_bin * 128
    s_total = nb * slots_per_bin

    order = np.lexsort((dst,))  # stable sort by dst => sorted by (core,bin)
    grp = core_of[order] * nb + bin_of[order]
    first = np.r_[True, grp[1:] != grp[:-1]]
    idx_of_first = np.maximum.accumulate(np.where(first, np.arange(len(grp)), 0))
    rank_in_bin = np.arange(len(grp)) - idx_of_first

    # --- shared weight pack ---
    # fc_w2 cols are (d,i,o); permute to (i,d,o) so messages/slab come out
    # as (c,d,o) — phase F can then merge (c,d) into one strided axis.
    w2p = fc_w2.reshape(64, 8, 8, 8).transpose(0, 2, 1, 3).reshape(64, 512)
    b2p = fc_b2.reshape(8, 8, 8).transpose(1, 0, 2).reshape(512)
    w1x = W_lin1.reshape(8, 128)  # [(i),(a,j)]
    w1s = (W_sc * C_S).reshape(8, 128)  # [(i),(a,o)]
    ssel = np.zeros((16, 8, 4, 4, 8), np.float32)
    for a in range(16):
        for j in range(8):
            for c in range(4):
                ssel[a, j, c, c, j] = 1.0
    ssel = ssel.reshape(128, 128)  # [(a,j),(c,c',j')]
    w2lr = (W_lin2 * (C_X * INV_SQRT_NEI)).transpose(1, 0, 2).reshape(16, 64)
    repa = np.zeros((16, 16, 8), np.float32)
    for a in range(16):
        repa[a, a, :] = 1.0
    repa = repa.reshape(16, 128)

    wpack = np.zeros((128, _WPACK_COLS), np.float32)
    wpack[0:128, 0:128] = ssel
    wpack[0:64, 128:640] = w2p
    wpack[64:65, 128:640] = b2p[None, :]
    wpack[0:64, 640:641] = fc_b1[:, None]
    wpack[65:97, 128:192] = fc_w1
    wpack[65:81, 192:320] = repa
    wpack[81:89, 192:320] = w1x
    wpack[89:97, 192:320] = w1s
    wpack[97:113, 192:256] = w2lr
    has_b2 = bool(np.any(fc_b2 != 0.0))

    in_maps = []
    for k in range(NCORES):
        lo = k * ns
        mask = core_of[order] == k
        slot = bin_of[order][mask] * slots_per_bin + rank_in_bin[mask]
        eidx = order[mask]
        lb = dst[eidx] - lo

        efT = np.zeros((32, s_total), np.float32)
        efT[:, slot] = edge_feat[eidx].T
        sv = src[eidx]
        esrc_flat = np.zeros(s_total, np.int32)
        esrc_flat[slot] = (sv // ns) * npad + (sv % ns)
        assert NCORES * npad <= 1 << 16
        # dst offset stored as the BITS of its float32 value (low 16 bits of
        # a small-int float32 are zero); esrc in the low 16 bits.
        dsto_flat = np.full(s_total, -1.0, np.float32)
        dsto_flat[slot] = (lb % nbin).astype(np.float32)
        epack_flat = dsto_flat.view(np.int32) | esrc_flat
        epackT = np.ascontiguousarray(epack_flat.reshape(-1, 128).T)  # [128,ntile]

        sl = slice(lo, lo + ns)
        bigT = np.zeros((32, npad + nh + s_total), np.float32)
        bigT[:, :ns] = node_input[sl].reshape(ns, 32).T
        attrT = np.zeros((16, npad), np.float32)
        attrT[:, :ns] = node_attr[sl].T
        bigT[0:16, npad : npad + nh] = attrT[:, :nh]
        bigT[16:32, npad : npad + nh] = attrT[:, nh:]
        bigT[:, npad + nh :] = efT

        # epack rides along in wpack's tail columns (bitcast i32 -> f32 bytes)
        wepack = np.concatenate([wpack, epackT.view(np.float32)], axis=1)
        in_maps.append({"bigT": _to_bf16(bigT), "wpack": wepack})
    return in_maps, tiles_per_bin, nb, npad, s_total, has_b2


# ---------------------------------------------------------------- device
def _build(T, nb, npad, s_total, has_b2, dbg=False):
    nh = npad // 2
    ntile = s_total // 128
    nc = bacc.Bacc("TRN2", debug=False, num_devices=NCORES)

    d_bigT = nc.dram_tensor(
        "bigT", [32, npad + nh + s_total], BF16, kind="ExternalInput"
    ).ap()
    d_wpack = nc.dram_tensor(
        "wpack", [128, _WPACK_COLS + ntile], F32, kind="ExternalInput"
    ).ap()
    d_out = nc.dram_tensor("out", [npad, 256], BF16, kind="ExternalOutput").ap()
    if dbg:
        d_dbg = {
            "dbg_xfull": nc.dram_tensor(
                "dbg_xfull", [NCORES * npad, 32], BF16, kind="ExternalOutput"
            ).ap(),
            "dbg_xg": nc.dram_tensor(
                "dbg_xg", [128, T * 32], BF16, kind="ExternalOutput"
            ).ap(),
            "dbg_ef": nc.dram_tensor(
                "dbg_ef", [128, 512], BF16, kind="ExternalOutput"
            ).ap(),
            "dbg_prod": nc.dram_tensor(
                "dbg_prod", [128, 2048], BF16, kind="ExternalOutput"
            ).ap(),
            "dbg_oh": nc.dram_tensor(
                "dbg_oh", [128, 128], BF16, kind="ExternalOutput"
            ).ap(),
            "dbg_sT": nc.dram_tensor(
                "dbg_sT", [32, npad], F32, kind="ExternalOutput"
            ).ap(),
            "dbg_esrc": nc.dram_tensor(
                "dbg_esrc", [128, ntile], I32, kind="ExternalOutput"
            ).ap(),
            "dbg_dsto": nc.dram_tensor(
                "dbg_dsto", [128, ntile], I32, kind="ExternalOutput"
            ).ap(),
        }

    mult = mybir.AluOpType.mult
    addop = mybir.AluOpType.add

    with TileContext(nc) as tc:
        with (
            tc.tile_pool(name="const", bufs=1) as const,
            tc.tile_pool(name="dram", bufs=1, space="DRAM") as dram,
        ):
            # ---- unpack constants: DMA f32 staging + bf16 conversion ----
            ident = const.tile([128, 128], F32)
            make_identity(nc, ident[:])

            def load_const(name, r0, r1, c0, c1, dtype=BF16):
                rows, cols = r1 - r0, c1 - c0
                stage = const.tile([rows, cols], F32, tag=f"stg_{name}")
                nc.sync.dma_start(out=stage[:], in_=d_wpack[r0:r1, c0:c1])
                if dtype == F32:
                    return stage
                t = const.tile([rows, cols], dtype, tag=f"cst_{name}")
                nc.scalar.copy(out=t[:], in_=stage[:])
                return t

            ssel_sb = load_const("ssel", 0, 128, 0, 128)
            w2p_sb = load_const("w2p", 0, 64, 128, 640)
            fcb1_sb = load_const("fcb1", 0, 64, 640, 641, dtype=F32)
            w1_sb = load_const("fcw1", 65, 97, 128, 192)
            repa_sb = load_const("repa", 65, 81, 192, 320)
            w1x_sb = load_const("w1x", 81, 89, 192, 320)
            w1s_sb = load_const("w1s", 89, 97, 192, 320)
            w2lr_sb = load_const("w2lr", 97, 113, 192, 256)
            if has_b2:
                b2p_sb = load_const("b2p", 64, 65, 128, 640)
                ones_sb = const.tile([1, 128], BF16)
                nc.vector.memset(ones_sb[:], 1.0)

            iota_sb = const.tile([128, 128], F32)
            nc.gpsimd.iota(
                iota_sb[:],
                pattern=[[1, 128]],
                base=0,
                channel_multiplier=0,
                allow_small_or_imprecise_dtypes=True,
            )

            attrT_sb = const.tile([16, npad], BF16)
            nc.sync.dma_start(
                out=attrT_sb[:, :nh], in_=d_bigT[0:16, npad : npad + nh]
            )
            nc.sync.dma_start(
                out=attrT_sb[:, nh:], in_=d_bigT[16:32, npad : npad + nh]
            )

            epack_sb = const.tile([128, ntile], I32)
            nc.sync.dma_start(
                out=epack_sb[:], in_=d_wpack[:, _WPACK_COLS:].bitcast(I32)
            )
            esrc_sb = const.tile([128, ntile], I32)
            nc.vector.tensor_scalar(
                out=esrc_sb[:],
                in0=epack_sb[:],
                scalar1=0xFFFF,
                scalar2=None,
                op0=mybir.AluOpType.bitwise_and,
            )
            dsto_sb = const.tile([128, ntile], I32)
            nc.vector.tensor_scalar(
                out=dsto_sb[:],
                in0=epack_sb[:],
                scalar1=-65536,  # 0xFFFF0000: keep the float32-bits payload
                scalar2=None,
                op0=mybir.AluOpType.bitwise_and,
            )

            sT_sb = const.tile([32, npad], F32)
            a2_all = const.tile([128, nb * 64], BF16)
            s_all = const.tile([128, nb * 32], BF16)

            x_shard = dram.tile([npad, 32], BF16)
            x_full = dram.tile([NCORES * npad, 32], BF16, addr_space="Shared")

            # ---------------- phase N: node linears ----------------
            chunks = []
            base = 0
            while base < npad:
                cw = min(512, npad - base)
                chunks.append((base, cw))
                base += cw
            with (
                tc.tile_pool(name="n1", bufs=3) as pn,
                tc.tile_pool(name="n1ps", bufs=2, space="PSUM") as pnps,
            ):
                for base, cw in chunks:
                    inT_cs = []
                    for c in range(4):
                        t = pn.tile([8, cw], BF16, tag=f"inT{c}")
                        nc.sync.dma_start(
                            out=t[:],
                            in_=d_bigT[c * 8 : (c + 1) * 8, base : base + cw],
                        )
                        inT_cs.append(t)
                    atr_ps = pnps.tile([128, cw], F32, tag="atrp", bufs=1)
                    nc.tensor.matmul(
                        out=atr_ps[:],
                        lhsT=repa_sb[:],
                        rhs=attrT_sb[:, base : base + cw],
                        start=True,
                        stop=True,
                    )
                    atr_sb = pn.tile([128, cw], F32, tag="atr")
                    nc.scalar.copy(out=atr_sb[:], in_=atr_ps[:])
                    xT_ps = pnps.tile([32, cw], F32, tag="xT", bufs=1)
                    sT_ps = pnps.tile([32, cw], F32, tag="sT", bufs=1)
                    for c in range(4):
                        rhs = inT_cs[c][:]
                        u_ps = pnps.tile([128, cw], F32, tag="u")
                        nc.tensor.matmul(
                            out=u_ps[:], lhsT=w1x_sb[:], rhs=rhs, start=True, stop=True
                        )
                        pr_sb = pn.tile([128, cw], BF16, tag="pr")
                        nc.vector.tensor_tensor(
                            out=pr_sb[:], in0=u_ps[:], in1=atr_sb[:], op=mult
                        )
                        nc.tensor.matmul(
                            out=xT_ps[:],
                            lhsT=ssel_sb[:, c * 32 : (c + 1) * 32],
                            rhs=pr_sb[:],
                            start=(c == 0),
                            stop=(c == 3),
                        )
                        u2_ps = pnps.tile([128, cw], F32, tag="u")
                        nc.tensor.matmul(
                            out=u2_ps[:], lhsT=w1s_sb[:], rhs=rhs, start=True, stop=True
                        )
                        pr2_sb = pn.tile([128, cw], BF16, tag="pr")
                        nc.vector.tensor_tensor(
                            out=pr2_sb[:], in0=u2_ps[:], in1=atr_sb[:], op=mult
                        )
                        nc.tensor.matmul(
                            out=sT_ps[:],
                            lhsT=ssel_sb[:, c * 32 : (c + 1) * 32],
                            rhs=pr2_sb[:],
                            start=(c == 0),
                            stop=(c == 3),
                        )
                    nc.scalar.copy(out=sT_sb[:, base : base + cw], in_=sT_ps[:])
                    xT_sb = pn.tile([32, cw], F32, tag="xTs")
                    nc.scalar.copy(out=xT_sb[:], in_=xT_ps[:])
                    for q in range(cw // 128):
                        xr_ps = pnps.tile([128, 32], F32, tag="xr")
                        nc.tensor.transpose(
                            out=xr_ps[:],
                            in_=xT_sb[:, q * 128 : (q + 1) * 128],
                            identity=ident[:32, :32],
                        )
                        xr_sb = pn.tile([128, 32], BF16, tag="xrs")
                        nc.scalar.copy(out=xr_sb[:], in_=xr_ps[:])
                        nc.sync.dma_start(
                            out=x_shard[base + q * 128 : base + (q + 1) * 128, :],
                            in_=xr_sb[:],
                        )

            # ---- precompute per-bin lin2 matrices + self-connection rows ----
            with (
                tc.tile_pool(name="pf0", bufs=2) as pf0,
                tc.tile_pool(name="pf0ps", bufs=2, space="PSUM") as pf0ps,
            ):
                for b in range(nb):
                    a2t_ps = pf0ps.tile([64, 128], F32, tag="a2t")
                    nc.tensor.matmul(
                        out=a2t_ps[:],
                        lhsT=w2lr_sb[:],
                        rhs=attrT_sb[:, b * 128 : (b + 1) * 128],
                        start=True,
                        stop=True,
                    )
                    a2t_sb = pf0.tile([64, 128], F32, tag="a2ts")
                    nc.scalar.copy(out=a2t_sb[:], in_=a2t_ps[:])
                    a2_ps = pf0ps.tile([128, 64], F32, tag="a2")
                    nc.tensor.transpose(
                        out=a2_ps[:], in_=a2t_sb[:], identity=ident[:64, :64]
                    )
                    nc.scalar.copy(
                        out=a2_all[:, b * 64 : (b + 1) * 64], in_=a2_ps[:]
                    )
                    s_ps = pf0ps.tile([128, 32], F32, tag="s")
                    nc.tensor.transpose(
                        out=s_ps[:],
                        in_=sT_sb[:, b * 128 : (b + 1) * 128],
                        identity=ident[:32, :32],
                    )
                    nc.scalar.copy(
                        out=s_all[:, b * 32 : (b + 1) * 32], in_=s_ps[:]
                    )

            # ---------------- allgather x ----------------
            nc.gpsimd.collective_compute(
                "AllGather",
                mybir.AluOpType.bypass,
                ins=[x_shard[:]],
                outs=[x_full[:]],
                replica_groups=[list(range(NCORES))],
            )
            if dbg:
                nc.sync.dma_start(out=d_dbg["dbg_xfull"][:], in_=x_full[:])
                nc.sync.dma_start(out=d_dbg["dbg_sT"][:], in_=sT_sb[:])
                nc.sync.dma_start(out=d_dbg["dbg_esrc"][:], in_=esrc_sb[:])
                nc.sync.dma_start(out=d_dbg["dbg_dsto"][:], in_=dsto_sb[:])

            # ---------------- phase E: edges ----------------
            ef_base = npad + nh
            with (
                tc.tile_pool(name="pe", bufs=3) as pe,
                tc.tile_pool(name="peps", bufs=2, space="PSUM") as peps,
            ):
                for b in range(nb):
                    efT_sb = pe.tile([32, T * 128], BF16, tag="efT")
                    nc.sync.dma_start(
                        out=efT_sb[:],
                        in_=d_bigT[
                            :, ef_base + b * T * 128 : ef_base + (b + 1) * T * 128
                        ],
                    )
                    # per-tile gathers of x[src] rows (multi-column offset APs
                    # return garbage on HW; [128,1]-offset form is proven)
                    xg_sb = pe.tile([128, T * 32], BF16, tag="xg", bufs=2)
                    for j in range(T):
                        nc.gpsimd.indirect_dma_start(
                            out=xg_sb[:, j * 32 : (j + 1) * 32],
                            out_offset=None,
                            in_=x_full[:],
                            in_offset=IndirectOffsetOnAxis(
                                ap=esrc_sb[:, b * T + j : b * T + j + 1], axis=0
                            ),
                        )
                    if dbg and b == 0:
                        nc.sync.dma_start(out=d_dbg["dbg_xg"][:], in_=xg_sb[:])
                    bin_ps = peps.tile([128, 256], F32, tag="bin")
                    for j in range(T):
                        t = b * T + j
                        # radial MLP layer 1
                        hT_ps = peps.tile([64, 128], F32, tag="hT")
                        nc.tensor.matmul(
                            out=hT_ps[:],
                            lhsT=w1_sb[:],
                            rhs=efT_sb[:, j * 128 : (j + 1) * 128],
                            start=True,
                            stop=True,
                        )
                        ha_sb = pe.tile([64, 128], BF16, tag="ha")
                        nc.scalar.activation(
                            out=ha_sb[:],
                            in_=hT_ps[:],
                            func=mybir.ActivationFunctionType.Silu,
                            bias=fcb1_sb[:],
                        )
                        # layer 2 -> ef [128e, (i,o,d)]
                        ef_ps = peps.tile([128, 512], F32, tag="ef")
                        nc.tensor.matmul(
                            out=ef_ps[:],
                            lhsT=ha_sb[:],
                            rhs=w2p_sb[:],
                            start=True,
                            stop=not has_b2,
                        )
                        if has_b2:
                            nc.tensor.matmul(
                                out=ef_ps[:],
                                lhsT=ones_sb[:],
                                rhs=b2p_sb[:],
                                start=False,
                                stop=True,
                            )
                        if dbg and t == 0:
                            ef_sb = pe.tile([128, 512], BF16, tag="efs")
                            nc.scalar.copy(out=ef_sb[:], in_=ef_ps[:])
                            nc.sync.dma_start(out=d_dbg["dbg_ef"][:], in_=ef_sb[:])
                        # bilinear messages: prod[e,(i,c,d,o)] = xg[e,(c,i)]*ef[e,(i,d,o)]
                        # (in0 reads the lin2 PSUM directly; no SBUF copy.
                        # The i-reduction happens in the 8 PSUM-accumulating
                        # scatter matmuls below — no DVE reduce needed.)
                        prod_sb = pe.tile([128, 2048], BF16, tag="prod")
                        nc.vector.tensor_tensor(
                            out=prod_sb[:].rearrange(
                                "p (i c m) -> p i c m", i=8, c=4
                            ),
                            in0=ef_ps[:]
                            .rearrange("p (i m) -> p i m", i=8)
                            .unsqueeze(2)
                            .to_broadcast((128, 8, 4, 64)),
                            in1=xg_sb[:, j * 32 : (j + 1) * 32]
                            .rearrange("p (c i) -> p i c", c=4)
                            .unsqueeze(3)
                            .to_broadcast((128, 8, 4, 64)),
                            op=mult,
                        )
                        # one-hot scatter matrix
                        oh_sb = pe.tile([128, 128], BF16, tag="oh")
                        nc.gpsimd.tensor_scalar(
                            out=oh_sb[:],
                            in0=iota_sb[:],
                            scalar1=dsto_sb[:, t : t + 1].bitcast(F32),
                            scalar2=None,
                            op0=mybir.AluOpType.is_equal,
                        )
                        if dbg and t == 0:
                            nc.sync.dma_start(
                                out=d_dbg["dbg_prod"][:], in_=prod_sb[:]
                            )
                            nc.sync.dma_start(out=d_dbg["dbg_oh"][:], in_=oh_sb[:])
                        for k in range(8):
                            nc.tensor.matmul(
                                out=bin_ps[:],
                                lhsT=oh_sb[:],
                                rhs=prod_sb[:, k * 256 : (k + 1) * 256],
                                start=(j == 0 and k == 0),
                                stop=(j == T - 1 and k == 7),
                            )
                    # ---- folded phase F: lin2 + self-connection, straight
                    # from this bin's PSUM accumulator (c,d,o) ----
                    prodF_sb = pe.tile([128, 2048], BF16, tag="prodF")
                    nc.vector.tensor_tensor(
                        out=prodF_sb[:].rearrange(
                            "p (q m o) -> p q m o", q=8, o=8
                        ),
                        in0=bin_ps[:]
                        .rearrange("p (m o) -> p m o", o=8)
                        .unsqueeze(1)
                        .to_broadcast((128, 8, 32, 8)),
                        in1=a2_all[:, b * 64 : (b + 1) * 64]
                        .rearrange("p (o q) -> p q o", o=8)
                        .unsqueeze(2)
                        .to_broadcast((128, 8, 32, 8)),
                        op=mult,
                    )
                    x2_sb = pe.tile([128, 256], F32, tag="x2")
                    nc.vector.tensor_reduce(
                        out=x2_sb[:],
                        in_=prodF_sb[:].rearrange("p (m o) -> p m o", o=8),
                        axis=mybir.AxisListType.X,
                        op=addop,
                    )
                    out_sb = pe.tile([128, 256], BF16, tag="outt")
                    # out[n,(q,c,d)] = x2 + s[n,(c,q)] broadcast over d
                    s_b = (
                        s_all[:, b * 32 : (b + 1) * 32]
                        .rearrange("p (c o) -> p o c", o=8)
                        .unsqueeze(3)
                        .to_broadcast((128, 8, 4, 8))
                    )
                    x2_r = x2_sb[:].rearrange("p (q c d) -> p q c d", c=4, d=8)
                    out_r = out_sb[:].rearrange("p (q c d) -> p q c d", c=4, d=8)
                    nc.vector.tensor_tensor(out=out_r, in0=x2_r, in1=s_b, op=addop)
                    nc.sync.dma_start(
                        out=d_out[b * 128 : (b + 1) * 128, :], in_=out_sb[:]
                    )

    nc.finalize()
    return nc


_BUILD_CACHE = {}


def _get_nc(T, nb, npad, s_total, has_b2):
    key = (T, nb, npad, s_total, has_b2)
    if key not in _BUILD_CACHE:
        _BUILD_CACHE[key] = _build(T, nb, npad, s_total, has_b2)
    return _BUILD_CACHE[key]


def kernel(**inputs):
    n = inputs["node_input"].shape[0]
    ns = n // NCORES
    in_maps, T, nb, npad, s_total, has_b2 = _host_prep(inputs, ns, 128)
    nc = _get_nc(T, nb, npad, s_total, has_b2)
    res = run_bass_kernel_spmd(nc, in_maps, list(range(NCORES)))
    # device rows are (q, c, d) in bf16; reference output is (c, d, q) f32
    out = np.concatenate(
        [
            np.asarray(res.results[k]["out"][:ns], np.float32)
            .reshape(ns, 8, 4, 8)
            .transpose(0, 2, 3, 1)
            for k in range(NCORES)
        ],
        axis=0,
    )
    return np.ascontiguousarray(out, np.float32)
